# revision 13
# baseline (speedup 1.0000x reference)
"""Spatio-temporal Hawkes process log-likelihood on Trainium2 (Bass/Tile).

Computes, for x[B, L, 3] = (t, s1, s2) and scalars mu/alpha/beta/sigma:
  lams[b, i]  = softplus(sum_{j<i} K(x_i, x_j) * 1[t_j>0] + mu) + 1e-5
  loglik[b]   = sum_i log(lams[b,i]) * 1[t_i>0]
              - UNIT_VOL * sum_{r, g} softplus(sum_j K((tt_r, ss_g), x_j) * m + mu)
with K(x, y) = norm * exp(-beta*(t_x - t_y) - |s_x - s_y|^2 / (2 sigma^2)),
norm = alpha*beta/(2 pi sigma^2), over a 50 x 50 x 50 (t, s1, s2) grid.

Strategy (one batch element per NeuronCore, 8 cores, data-parallel):
  The grid kernel factorizes: exp(-beta*(tt_r - t_j)) * exp(-ds2/2sig^2).
  Per core build G[j, g] = exp(-inv2sig2 * |ss_g - s_j|^2)  (via a K=5
  matmul computing the quadratic expansion of ds2, the per-event s^2
  term riding in the ACT bias), and
  W[j, r] = norm * 1[0 < t_j <= tt_r] * exp(beta*(t_j - tt_r)).
  Then softplus-arg = W.T @ G on the PE (bf16 operands - the outputs
  only feed softplus+sum, fp32 accumulation in PSUM), with the
  softplus+row-sum stage chunk-pipelined against the matmuls.
  The per-event [L, L] exponent is built by 4 accumulated rank-1 fp32
  matmuls (outer sums + cross terms; fp32 because the expansion
  cancels catastrophically in low precision), one ACT exp, and a
  masked row-reduce fused into a scalar_tensor_tensor.

  Partition packing: the 2500 spatial grid points are split in two
  halves of 1250; partitions 0:64 hold events-vs-half0, 64:128 hold
  events-vs-half1, so elementwise engines run at full 128-lane width.

softplus is decomposed as relu(v) + log1p(exp(-|v|)) with
|v| = 2*relu(v) - v (DVE) so the only ACT funcs are Exp/Ln/Copy; the
activation-table map is patched during compile so every func resolves
to the single `natural_log_exp_and_others` set -> one table load.

All tiny per-core staging (dup columns, concatenated rows, the K=5
lhsT) is marshalled host-side as pure copies - engines can only
address SBUF partition starts of 0/32/64/96, so single-row writes at
other partitions are not expressible on-device.
"""

import math
import numpy as np
from contextlib import ExitStack

R = 50                      # INT_RES (time and each spatial axis)
RG = R * R                  # 2500 spatial grid points
HALF = RG // 2              # 1250
NCORES = 8
UNIT_VOL = 1.0 / float(R ** 3)
BIG_NEG = 1.0e30
CHUNKS = ((0, 512), (512, 512), (1024, HALF - 1024))

_prog_cache: dict = {}


def _const_arrays(L: int, norm: float, beta: float, inv2sig2: float):
    f32 = np.float32
    g1 = np.linspace(0.0, 1.0, R).astype(f32)
    g2 = np.linspace(0.0, 1.0, R).astype(f32)

    # const blob [128, 384]:
    #   [0:64, 0:64]   tril_n (norm * strict lower triangular)
    #   [:, 64:65]     ones column
    #   [:, 65:66]     sel column (valid packed r rows)
    #   [0:1, 66:194]  ones row (128)
    #   [0:1, 194:244] -linspace(0,1,R)
    #   [0:2, 244:246] per-event STT coefficient columns
    #   [0:4, 256:281] rhs of the A-table matmul [g1lo^2; g1hi^2; -2g1lo; -2g1hi]
    #   [0:2, 288:338] rhs of the B-table matmul [g2^2; -2g2]
    cblob = np.zeros((128, 384), f32)
    cblob[0:L, 0:L] = norm * np.tril(np.ones((L, L), np.float64), -1)
    cblob[:, 64] = 1.0
    cblob[:, 65] = (np.arange(128) % 64 < R)
    cblob[0, 66:194] = 1.0
    cblob[0, 194:244] = -np.linspace(0.0, 1.0, R)
    cblob[0, 244] = -beta; cblob[1, 244] = 2.0 * inv2sig2   # colA -> [u; a1]
    cblob[0, 245] = beta;  cblob[1, 245] = 2.0 * inv2sig2   # colB -> [v; a2]
    g1lo, g1hi = g1[0:25], g1[25:50]
    cblob[0, 256:281] = g1lo ** 2
    cblob[1, 256:281] = g1hi ** 2
    cblob[2, 256:281] = -2.0 * g1lo
    cblob[3, 256:281] = -2.0 * g1hi
    cblob[0, 288:338] = g2 ** 2
    cblob[1, 288:338] = -2.0 * g2
    return dict(cblob=cblob)


def _marshal_core_inputs(t, s1, s2):
    """Pure-layout staging of one sequence's inputs (no arithmetic).

    iblob [128, 600]:
      [:, 0:3]       t/s1/s2 duplicated into both partition halves
      [0:1, 3:195]   t | s1 | s2 concatenated rows
      [0:5, 195:323] K=5 lhsT for the ds2 matmul:
                     [ind_lo; ind_hi; s1*ind_lo; s1*ind_hi; s2_dup]
      [0:2, 336:400] [t; s1]   (pair-packed per-event row inputs)
      [0:2, 400:464] [t; s2]
      [0:2, 464:528] [ones; s1] (rhs of per-event matmul 1)
      [0:2, 528:592] [ones; s2] (lhsT of per-event matmul 2)
      [0:2, 600:728] [ones128; s2_dup] (lhsT of the B-table matmul)
    """
    f32 = np.float32
    L = t.shape[0]
    blob = np.zeros((128, 728), f32)
    blob[0:L, 0] = t; blob[64:64 + L, 0] = t
    blob[0:L, 1] = s1; blob[64:64 + L, 1] = s1
    blob[0:L, 2] = s2; blob[64:64 + L, 2] = s2
    blob[0, 3:3 + L] = t
    blob[0, 3 + L:3 + 2 * L] = s1
    blob[0, 3 + 2 * L:3 + 3 * L] = s2
    blob[0, 195:195 + 64] = 1.0                        # ind_lo
    blob[1, 195 + 64:195 + 128] = 1.0                  # ind_hi
    blob[2, 195:195 + L] = s1
    blob[3, 195 + 64:195 + 64 + L] = s1
    blob[4, 195:195 + L] = s2
    blob[4, 195 + 64:195 + 64 + L] = s2
    blob[0, 336:336 + L] = t;    blob[1, 336:336 + L] = s1
    blob[0, 400:400 + L] = t;    blob[1, 400:400 + L] = s2
    blob[0, 464:464 + L] = 1.0;  blob[1, 464:464 + L] = s1
    blob[0, 528:528 + L] = 1.0;  blob[1, 528:528 + L] = s2
    blob[0, 600:728] = 1.0
    blob[1, 600:600 + L] = s2; blob[1, 664:664 + L] = s2
    return {"iblob": blob}


def _patched_act_tables(orig_fn, preferred="natural_log_exp_and_others"):
    """Wrap get_activation_tables so every function present in the
    preferred set resolves only to it (same names/order, so the emitted
    act_func_set_id still indexes the real act_info.json)."""
    import functools

    @functools.cache
    def wrapper(arch):
        tables = dict(orig_fn(arch))
        pref = tables.get(preferred)
        if not pref:
            return tables
        return {
            name: (funcs if name == preferred else funcs - pref)
            for name, funcs in tables.items()
        }
    return wrapper


def _build_program(mu: float, beta: float, inv2sig2: float, norm: float, L: int):
    import concourse.bass as bass
    import concourse.bacc as bacc
    import concourse.tile as tile
    import concourse.mybir as mybir

    f32 = mybir.dt.float32
    f32r = mybir.dt.float32r
    bf16 = mybir.dt.bfloat16
    Act = mybir.ActivationFunctionType
    Op = mybir.AluOpType

    nc = bacc.Bacc("TRN2", target_bir_lowering=False, debug=False,
                   enable_asserts=True, num_devices=NCORES)

    # ---- DRAM I/O
    iblob_d = nc.dram_tensor("iblob", [128, 728], f32, kind="ExternalInput").ap()
    cblob_d = nc.dram_tensor("cblob", [128, 384], f32, kind="ExternalInput").ap()
    lams_o = nc.dram_tensor("lams_o", [L], f32, kind="ExternalOutput").ap()
    ll_o = nc.dram_tensor("ll_o", [1], f32, kind="ExternalOutput").ap()

    with tile.TileContext(nc) as tc, ExitStack() as ctx:
        pool = ctx.enter_context(tc.tile_pool(name="sbuf", bufs=1))
        cpool = ctx.enter_context(tc.tile_pool(name="chunk", bufs=2))
        psum = ctx.enter_context(tc.tile_pool(name="psum", bufs=1,
                                              space=bass.MemorySpace.PSUM))
        psmall = ctx.enter_context(tc.tile_pool(name="psmall", bufs=3,
                                                space=bass.MemorySpace.PSUM))

        # ---- loads (2 DMAs on 2 queues)
        iblob = pool.tile([128, 728], f32)
        nc.sync.dma_start(iblob[:], iblob_d[:])
        cblob = pool.tile([128, 384], f32)
        nc.gpsimd.dma_start(cblob[:], cblob_d[:])

        t_col = iblob[:, 0:1]
        s1_col = iblob[:, 1:2]
        s2_col = iblob[:, 2:3]
        t_row = iblob[0:1, 3:3 + L]
        s1_row = iblob[0:1, 3 + L:3 + 2 * L]
        s2_row = iblob[0:1, 3 + 2 * L:3 + 3 * L]
        lhsT4A = iblob[0:4, 195:323]
        lhsT2B = iblob[0:2, 600:728]

        tril = cblob[0:L, 0:L]
        ones_col = cblob[:, 64:65]
        sel_col = cblob[:, 65:66]
        ones_r = cblob[0:1, 66:194]
        ones_r64 = cblob[0:1, 66:130]
        negttg = cblob[0:1, 194:244]
        scoefA = cblob[0:2, 244:245]
        scoefB = cblob[0:2, 245:246]
        rhsA = cblob[0:4, 256:281]
        rhsB = cblob[0:2, 288:338]

        mu_col = pool.tile([128, 1], f32)
        nc.vector.memset(mu_col[:], mu)

        # ---- per-partition spatial biases: -inv2sig2 * s1^2, -inv2sig2 * s2^2
        biasA = pool.tile([128, 1], f32)
        nc.vector.tensor_scalar(biasA[:], s1_col, s1_col, -inv2sig2,
                                Op.mult, Op.mult)
        biasB = pool.tile([128, 1], f32)
        nc.vector.tensor_scalar(biasB[:], s2_col, s2_col, -inv2sig2,
                                Op.mult, Op.mult)

        # ---- temporal weights W_T[j(packed), r] (norm folded into mask)
        bc_ps = psmall.tile([128, R], f32, tag="small")
        nc.tensor.matmul(bc_ps[:], ones_r[:], negttg[:], start=True, stop=True)
        dtW = pool.tile([128, R], f32)
        nc.vector.tensor_scalar(dtW[:], bc_ps[:], t_col, None, Op.add)
        Ew = pool.tile([128, R], f32)
        nc.scalar.activation(Ew[:], dtW[:], Act.Exp, scale=beta)
        hn_col = pool.tile([128, 1], f32)
        nc.vector.tensor_scalar(hn_col[:], t_col, 0.0, norm, Op.is_gt, Op.mult)
        h_col = pool.tile([128, 1], f32)
        nc.vector.tensor_scalar(h_col[:], t_col, 0.0, None, Op.is_gt)
        Mw = pool.tile([128, R], f32)
        nc.vector.tensor_scalar(Mw[:], dtW[:], 0.0, hn_col[:, 0:1], Op.is_le, Op.mult)
        WT2 = pool.tile([128, 64], bf16)
        nc.vector.memset(WT2[:], 0.0)
        nc.vector.tensor_tensor(WT2[:, 0:R], Ew[:], Mw[:], Op.mult)

        # ---- per-event exponent via 2 accumulated K=2 matmuls (fp32)
        pA_in = iblob[0:2, 336:336 + L]
        pB_in = iblob[0:2, 400:400 + L]
        rhs_mm1 = iblob[0:2, 464:464 + L]
        lhsT_mm2 = iblob[0:2, 528:528 + L]

        sq1 = pool.tile([1, L], f32)
        nc.vector.tensor_tensor(sq1[:], s1_row, s1_row, Op.mult)
        sq2 = pool.tile([1, L], f32)
        nc.vector.tensor_tensor(sq2[:], s2_row, s2_row, Op.mult)
        ssum = pool.tile([1, L], f32)
        nc.vector.tensor_tensor(ssum[:], sq1[:], sq2[:], Op.add)
        w2 = pool.tile([2, L], f32)
        nc.vector.memset(w2[:], 0.0)
        nc.vector.tensor_scalar(w2[0:1, :], ssum[:], -inv2sig2, None, Op.mult)

        pairA = pool.tile([2, L], f32)
        nc.vector.scalar_tensor_tensor(pairA[:], pA_in, scoefA,
                                       w2[:], Op.mult, Op.add)
        pairB = pool.tile([2, L], f32)
        nc.vector.scalar_tensor_tensor(pairB[:], pB_in, scoefB,
                                       w2[:], Op.mult, Op.add)
        ha = pool.tile([1, L], f32)
        nc.vector.tensor_scalar(ha[:], t_row, 0.0, 1.0, Op.is_gt, Op.subtract)
        hm = pool.tile([1, L], f32)
        nc.vector.tensor_scalar(hm[:], ha[:], BIG_NEG, None, Op.mult)
        nc.vector.tensor_tensor(pairB[0:1, :], pairB[0:1, :], hm[:], Op.add)

        zev_ps = psmall.tile([L, L], f32, tag="small")
        nc.tensor.matmul(zev_ps[:], pairA[:], rhs_mm1, start=True, stop=False)
        nc.tensor.matmul(zev_ps[:], lhsT_mm2, pairB[:], start=False, stop=True)

        Ke = pool.tile([L, L], f32)
        nc.scalar.activation(Ke[:], zev_ps[:], Act.Exp)
        Km = pool.tile([L, L], f32)
        lam_col = pool.tile([L, 1], f32)
        nc.vector.scalar_tensor_tensor(Km[:], Ke[:], 0.0, tril,
                                       Op.add, Op.mult, accum_out=lam_col[:])

        # lams = softplus(lam_raw + mu) + 1e-5 ; log-lik pieces
        rr = pool.tile([L, 1], f32)
        nc.vector.tensor_scalar(rr[:], lam_col[:], mu, 0.0, Op.add, Op.max)
        tt_ = pool.tile([L, 1], f32)
        nc.vector.scalar_tensor_tensor(tt_[:], rr[:], 2.0, lam_col[:],
                                       Op.mult, Op.subtract)
        ee = pool.tile([L, 1], f32)
        nc.scalar.activation(ee[:], tt_[:], Act.Exp, scale=-1.0,
                             bias=mu_col[0:L, 0:1])
        lp = pool.tile([L, 1], f32)
        nc.scalar.activation(lp[:], ee[:], Act.Ln, bias=1.0)
        lams_col = pool.tile([L, 1], f32)
        nc.vector.scalar_tensor_tensor(lams_col[:], lp[:], 1e-5, rr[:],
                                       Op.add, Op.add)
        nc.sync.dma_start(lams_o.unsqueeze(1), lams_col[:])

        lnl = pool.tile([L, 1], f32)
        nc.scalar.activation(lnl[:], lams_col[:], Act.Ln)
        lnm = pool.tile([L, 1], f32)
        nc.vector.tensor_tensor(lnm[:], lnl[:], h_col[0:L, 0:1], Op.mult)
        red_ps = psmall.tile([1, 2], f32, tag="small")
        nc.tensor.matmul(red_ps[0:1, 0:1], lnm[:, 0:1], ones_col[0:L, 0:1],
                         start=True, stop=True)

        # ---- separable spatial kernel: G[p, i*50+k] = A[p, i] * B[p, k]
        psA = psmall.tile([128, 32], f32, tag="small")
        nc.tensor.matmul(psA[:, 0:25], lhsT4A, rhsA, start=True, stop=True)
        A2 = pool.tile([128, 25], f32)
        nc.scalar.activation(A2[:], psA[:, 0:25], Act.Exp,
                             scale=-inv2sig2, bias=biasA[:, 0:1])
        psB = psmall.tile([128, 64], f32, tag="small")
        nc.tensor.matmul(psB[:, 0:R], lhsT2B, rhsB, start=True, stop=True)
        B2 = pool.tile([128, R], f32)
        nc.scalar.activation(B2[:], psB[:, 0:R], Act.Exp,
                             scale=-inv2sig2, bias=biasB[:, 0:1])
        G = pool.tile([128, HALF], bf16)
        nc.vector.tensor_tensor(
            G[:].rearrange("p (a b) -> p a b", a=25),
            A2[:].unsqueeze(2).broadcast_to([128, 25, R]),
            B2[:].unsqueeze(1).broadcast_to([128, 25, R]),
            Op.mult)

        # ---- grid: per 512-chunk pipeline of W.T@G -> softplus+accumulate
        acc = pool.tile([128, 4], f32)
        z_ps = psum.tile([128, 1536], f32, tag="z")
        for ci, (off, w) in enumerate(CHUNKS):
            for h in (0, 1):
                p0 = h * 64
                nc.tensor.matmul(z_ps[p0:p0 + 64, off:off + w],
                                 WT2[p0:p0 + 64, 0:64],
                                 G[p0:p0 + 64, off:off + w],
                                 start=True, stop=True)
            # softplus(v)+sum, v = z+mu: r=relu(v); exp(-|v|); log1p; +r
            r_c = cpool.tile([128, 512], f32, tag="r_c")
            nc.vector.tensor_scalar(r_c[:, 0:w], z_ps[:, off:off + w],
                                    mu, 0.0, Op.add, Op.max)
            a_c = cpool.tile([128, 512], f32, tag="a_c")
            nc.scalar.activation(a_c[:, 0:w], z_ps[:, off:off + w], Act.Abs,
                                 bias=mu_col[:, 0:1])
            e_c = cpool.tile([128, 512], f32, tag="e_c")
            nc.scalar.activation(e_c[:, 0:w], a_c[:, 0:w], Act.Exp,
                                 scale=-1.0)
            l_c = cpool.tile([128, 512], f32, tag="l_c")
            nc.scalar.activation(l_c[:, 0:w], e_c[:, 0:w], Act.Ln, bias=1.0)
            sp_c = cpool.tile([128, 512], f32, tag="sp_c")
            nc.vector.scalar_tensor_tensor(sp_c[:, 0:w], l_c[:, 0:w], 0.0,
                                           r_c[:, 0:w], Op.add, Op.add,
                                           accum_out=acc[:, ci:ci + 1])

        int_col = pool.tile([128, 1], f32)
        nc.vector.tensor_reduce(int_col[:], acc[:, 0:3],
                                mybir.AxisListType.X, Op.add)
        nc.tensor.matmul(red_ps[0:1, 1:2], int_col[:, 0:1], sel_col,
                         start=True, stop=True)

        sl_sb = pool.tile([1, 1], f32)
        nc.scalar.copy(sl_sb[:], red_ps[0:1, 0:1])
        ll_sb = pool.tile([1, 1], f32)
        nc.vector.scalar_tensor_tensor(ll_sb[:], red_ps[0:1, 1:2], -UNIT_VOL,
                                       sl_sb[:], Op.mult, Op.add)
        nc.sync.dma_start(ll_o.unsqueeze(1), ll_sb[:])

    import concourse.hw_specs as hw_specs
    orig = bacc.get_activation_tables
    bacc.get_activation_tables = _patched_act_tables(hw_specs.get_activation_tables)
    try:
        nc.compile()
    finally:
        bacc.get_activation_tables = orig
    return nc


def _get_program(mu, alpha, beta, sigma, L):
    key = (float(mu), float(alpha), float(beta), float(sigma), L)
    if key not in _prog_cache:
        sig2 = float(sigma) * float(sigma)
        inv2sig2 = 1.0 / (2.0 * sig2)
        norm = float(alpha) * float(beta) / (2.0 * math.pi * sig2)
        nc = _build_program(float(mu), float(beta), inv2sig2, norm, L)
        consts = _const_arrays(L, norm, float(beta), inv2sig2)
        _prog_cache[key] = (nc, consts)
    return _prog_cache[key]


def kernel(x, mu, alpha, beta, sigma):
    from concourse.bass_utils import run_bass_kernel_spmd

    x = np.asarray(x, dtype=np.float32)
    B, L, _ = x.shape
    assert B == NCORES, f"expected batch {NCORES}, got {B}"

    nc, consts = _get_program(mu, alpha, beta, sigma, L)

    in_maps = []
    for b in range(B):
        m = dict(consts)
        m.update(_marshal_core_inputs(x[b, :, 0], x[b, :, 1], x[b, :, 2]))
        in_maps.append(m)

    res = run_bass_kernel_spmd(nc, in_maps, list(range(NCORES)))
    lams = np.stack([res.results[b]["lams_o"] for b in range(B)]).astype(np.float32)
    loglik = np.stack([res.results[b]["ll_o"][0] for b in range(B)]).astype(np.float32)
    return lams, loglik


# revision 15
# speedup vs baseline: 1.2444x; 1.2444x over previous
"""Spatio-temporal Hawkes process log-likelihood on Trainium2 (Bass/Tile).

Computes, for x[B, L, 3] = (t, s1, s2) and scalars mu/alpha/beta/sigma:
  lams[b, i]  = softplus(sum_{j<i} K(x_i, x_j) * 1[t_j>0] + mu) + 1e-5
  loglik[b]   = sum_i log(lams[b,i]) * 1[t_i>0]
              - UNIT_VOL * sum_{r, g} softplus(sum_j K((tt_r, ss_g), x_j) * m + mu)
with K(x, y) = norm * exp(-beta*(t_x - t_y) - |s_x - s_y|^2 / (2 sigma^2)),
norm = alpha*beta/(2 pi sigma^2), over a 50 x 50 x 50 (t, s1, s2) grid.

Strategy (one batch element per NeuronCore, 8 cores, data-parallel):
  The grid kernel factorizes in time AND is separable in space:
    K((tt_r, ss_g), x_j) = [norm * 1[0<t_j<=tt_r] * e^{beta(t_j-tt_r)}]
                         * e^{-inv2sig2 (g1_i-s1_j)^2} * e^{-inv2sig2 (g2_k-s2_j)^2}
  so per core we build two tiny per-axis tables A[j,i] (25 cols/half)
  and B[j,k] (50 cols) via K<=4 quadratic-expansion matmuls + ACT exp,
  expand G[j,g] = A*B with one broadcast-AP multiply, build the
  temporal weight matrix W[j,r], and get the softplus argument as
  W.T @ G on the PE (bf16 operands, fp32 PSUM accumulation).
  softplus+row-sum is chunk-pipelined against the matmuls, with the
  row sums riding the accum_out ports of the DVE relu and ACT log1p.
  The per-event [L, L] exponent is built by 2 accumulated K=2 fp32
  matmuls over pair-packed rank-1 factors (fp32 because the quadratic
  expansion cancels catastrophically in low precision), one ACT exp,
  and a masked row-reduce fused into a scalar_tensor_tensor.

  Partition packing: the 2500 spatial grid points are split in two
  halves of 1250; partitions 0:64 hold events-vs-half0, 64:128 hold
  events-vs-half1, so elementwise engines run at full 128-lane width.

softplus is decomposed as relu(v) + log1p(exp(-|v|)) and the
activation-table map is patched during compile so every ACT func
(Exp/Ln/Abs/Copy) resolves to the single `natural_log_exp_and_others`
set -> one table load, no set thrashing.

All tiny per-core staging (dup columns, concatenated rows, packed
rank-1 factors) is marshalled host-side as pure copies - engines can
only address SBUF partition starts of 0/32/64/96, so single-row writes
at other partitions are not expressible on-device.  Emission order is
tuned so no in-order engine stream blocks on a late dependency.
"""

import math
import numpy as np
from contextlib import ExitStack

R = 50                      # INT_RES (time and each spatial axis)
RG = R * R                  # 2500 spatial grid points
HALF = RG // 2              # 1250
NCORES = 8
UNIT_VOL = 1.0 / float(R ** 3)
BIG_NEG = 1.0e30
CHUNKS = ((0, 512), (512, 512), (1024, HALF - 1024))

_prog_cache: dict = {}


def _const_arrays(L: int, norm: float, beta: float, inv2sig2: float):
    f32 = np.float32
    g1 = np.linspace(0.0, 1.0, R).astype(f32)
    g2 = np.linspace(0.0, 1.0, R).astype(f32)

    ccols = np.zeros((128, 2), f32)
    ccols[:, 0] = 1.0                                   # ones column
    ccols[:, 1] = (np.arange(128) % 64 < R)             # sel (valid r rows)

    ctril = (norm * np.tril(np.ones((L, L), np.float64), -1)).astype(f32)

    # crows [4, 256]:
    #   [0:1, 0:128]   ones row
    #   [0:1, 128:178] -linspace(0,1,R)
    #   [0:2, 178:180] per-event STT coefficient columns
    #   [0:4, 180:205] rhs of A-table matmul [g1lo^2; g1hi^2; -2g1lo; -2g1hi]
    #   [0:2, 205:255] rhs of B-table matmul [g2^2; -2g2]
    crows = np.zeros((4, 256), f32)
    crows[0, 0:128] = 1.0
    crows[0, 128:178] = -np.linspace(0.0, 1.0, R)
    crows[0, 178] = -beta; crows[1, 178] = 2.0 * inv2sig2    # colA -> [u; a1]
    crows[0, 179] = beta;  crows[1, 179] = 2.0 * inv2sig2    # colB -> [v; a2]
    g1lo, g1hi = g1[0:25], g1[25:50]
    crows[0, 180:205] = g1lo ** 2
    crows[1, 180:205] = g1hi ** 2
    crows[2, 180:205] = -2.0 * g1lo
    crows[3, 180:205] = -2.0 * g1hi
    crows[0, 205:255] = g2 ** 2
    crows[1, 205:255] = -2.0 * g2
    return dict(ccols=ccols, ctril=ctril, crows=crows)


def _marshal_core_inputs(t, s1, s2):
    """Pure-layout staging of one sequence's inputs (no arithmetic).

    icols [128, 3]: t/s1/s2 duplicated into both partition halves.
    irows [8, 704]:
      [0:1, 0:192]   t | s1 | s2 concatenated rows
      [0:4, 192:320] lhsT of the A-table matmul
                     [ind_lo; ind_hi; s1*ind_lo; s1*ind_hi]
      [0:2, 320:384] [t; s1]   (pair-packed per-event row inputs)
      [0:2, 384:448] [t; s2]
      [0:2, 448:512] [ones; s1] (rhs of per-event matmul 1)
      [0:2, 512:576] [ones; s2] (lhsT of per-event matmul 2)
      [0:2, 576:704] [ones128; s2_dup] (lhsT of the B-table matmul)
    """
    f32 = np.float32
    L = t.shape[0]
    icols = np.zeros((128, 3), f32)
    icols[0:L, 0] = t; icols[64:64 + L, 0] = t
    icols[0:L, 1] = s1; icols[64:64 + L, 1] = s1
    icols[0:L, 2] = s2; icols[64:64 + L, 2] = s2
    irows = np.zeros((8, 704), f32)
    irows[0, 0:L] = t
    irows[0, L:2 * L] = s1
    irows[0, 2 * L:3 * L] = s2
    irows[0, 192:256] = 1.0                       # ind_lo
    irows[1, 256:320] = 1.0                       # ind_hi
    irows[2, 192:192 + L] = s1
    irows[3, 256:256 + L] = s1
    irows[0, 320:320 + L] = t;   irows[1, 320:320 + L] = s1
    irows[0, 384:384 + L] = t;   irows[1, 384:384 + L] = s2
    irows[0, 448:448 + L] = 1.0; irows[1, 448:448 + L] = s1
    irows[0, 512:512 + L] = 1.0; irows[1, 512:512 + L] = s2
    irows[0, 576:704] = 1.0
    irows[1, 576:576 + L] = s2;  irows[1, 640:640 + L] = s2
    return {"icols": icols, "irows": irows}


def _patched_act_tables(orig_fn, preferred="natural_log_exp_and_others"):
    """Wrap get_activation_tables so every function present in the
    preferred set resolves only to it (same names/order, so the emitted
    act_func_set_id still indexes the real act_info.json)."""
    import functools

    @functools.cache
    def wrapper(arch):
        tables = dict(orig_fn(arch))
        pref = tables.get(preferred)
        if not pref:
            return tables
        return {
            name: (funcs if name == preferred else funcs - pref)
            for name, funcs in tables.items()
        }
    return wrapper


def _build_program(mu: float, beta: float, inv2sig2: float, norm: float, L: int):
    import concourse.bass as bass
    import concourse.bacc as bacc
    import concourse.tile as tile
    import concourse.mybir as mybir

    f32 = mybir.dt.float32
    bf16 = mybir.dt.bfloat16
    Act = mybir.ActivationFunctionType
    Op = mybir.AluOpType

    nc = bacc.Bacc("TRN2", target_bir_lowering=False, debug=False,
                   enable_asserts=True, num_devices=NCORES)

    # ---- DRAM I/O
    icols_d = nc.dram_tensor("icols", [128, 3], f32, kind="ExternalInput").ap()
    irows_d = nc.dram_tensor("irows", [8, 704], f32, kind="ExternalInput").ap()
    ccols_d = nc.dram_tensor("ccols", [128, 2], f32, kind="ExternalInput").ap()
    ctril_d = nc.dram_tensor("ctril", [L, L], f32, kind="ExternalInput").ap()
    crows_d = nc.dram_tensor("crows", [4, 256], f32, kind="ExternalInput").ap()
    lams_o = nc.dram_tensor("lams_o", [L], f32, kind="ExternalOutput").ap()
    ll_o = nc.dram_tensor("ll_o", [1], f32, kind="ExternalOutput").ap()

    with tile.TileContext(nc) as tc, ExitStack() as ctx:
        pool = ctx.enter_context(tc.tile_pool(name="sbuf", bufs=1))
        cpool = ctx.enter_context(tc.tile_pool(name="chunk", bufs=2))
        psum = ctx.enter_context(tc.tile_pool(name="psum", bufs=1,
                                              space=bass.MemorySpace.PSUM))
        psmall = ctx.enter_context(tc.tile_pool(name="psmall", bufs=3,
                                                space=bass.MemorySpace.PSUM))

        # ---- loads (5 small DMAs on 3 queues)
        irows = pool.tile([8, 704], f32)
        nc.sync.dma_start(irows[:], irows_d[:])
        icols = pool.tile([128, 3], f32)
        nc.sync.dma_start(icols[:], icols_d[:])
        crows = pool.tile([4, 256], f32)
        nc.gpsimd.dma_start(crows[:], crows_d[:])
        ctril = pool.tile([L, L], f32)
        nc.gpsimd.dma_start(ctril[:], ctril_d[:])
        ccols = pool.tile([128, 2], f32)
        nc.scalar.dma_start(ccols[:], ccols_d[:])

        t_col = icols[:, 0:1]
        s1_col = icols[:, 1:2]
        s2_col = icols[:, 2:3]
        t_row = irows[0:1, 0:L]
        s1_row = irows[0:1, L:2 * L]
        s2_row = irows[0:1, 2 * L:3 * L]
        lhsT4A = irows[0:4, 192:320]
        pA_in = irows[0:2, 320:320 + L]
        pB_in = irows[0:2, 384:384 + L]
        rhs_mm1 = irows[0:2, 448:448 + L]
        lhsT_mm2 = irows[0:2, 512:512 + L]
        lhsT2B = irows[0:2, 576:704]

        ones_col = ccols[:, 0:1]
        sel_col = ccols[:, 1:2]
        ones_r = crows[0:1, 0:128]
        ones_r64 = crows[0:1, 0:L]
        negttg = crows[0:1, 128:178]
        scoefA = crows[0:2, 178:179]
        scoefB = crows[0:2, 179:180]
        rhsA = crows[0:4, 180:205]
        rhsB = crows[0:2, 205:255]

        mu_col = pool.tile([128, 1], f32)
        nc.vector.memset(mu_col[:], mu)

        # ---- per-partition spatial biases: -inv2sig2*s1^2, -inv2sig2*s2^2
        biasA = pool.tile([128, 1], f32)
        nc.vector.tensor_scalar(biasA[:], s1_col, s1_col, -inv2sig2,
                                Op.mult, Op.mult)
        biasB = pool.tile([128, 1], f32)
        nc.vector.tensor_scalar(biasB[:], s2_col, s2_col, -inv2sig2,
                                Op.mult, Op.mult)

        # ---- temporal weights W_T[j(packed), r] (norm folded into mask)
        bc_ps = psmall.tile([128, R], f32, tag="small")
        nc.tensor.matmul(bc_ps[:], ones_r, negttg, start=True, stop=True)
        dtW = pool.tile([128, R], f32)
        nc.vector.tensor_scalar(dtW[:], bc_ps[:], t_col, None, Op.add)
        Ew = pool.tile([128, R], f32)
        nc.scalar.activation(Ew[:], dtW[:], Act.Exp, scale=beta)
        hn_col = pool.tile([128, 1], f32)
        nc.vector.tensor_scalar(hn_col[:], t_col, 0.0, norm, Op.is_gt, Op.mult)
        h_col = pool.tile([128, 1], f32)
        nc.vector.tensor_scalar(h_col[:], t_col, 0.0, None, Op.is_gt)
        Mw = pool.tile([128, R], f32)
        nc.vector.tensor_scalar(Mw[:], dtW[:], 0.0, hn_col[:, 0:1], Op.is_le, Op.mult)
        WT2 = pool.tile([128, 64], bf16)
        nc.vector.memset(WT2[:], 0.0)
        nc.vector.tensor_tensor(WT2[:, 0:R], Ew[:], Mw[:], Op.mult)

        # ---- separable spatial kernel: G[p, i*50+k] = A[p, i] * B[p, k]
        psA = psmall.tile([128, 32], f32, tag="small")
        nc.tensor.matmul(psA[:, 0:25], lhsT4A, rhsA, start=True, stop=True)
        A2 = pool.tile([128, 25], f32)
        nc.scalar.activation(A2[:], psA[:, 0:25], Act.Exp,
                             scale=-inv2sig2, bias=biasA[:, 0:1])
        psB = psmall.tile([128, 64], f32, tag="small")
        nc.tensor.matmul(psB[:, 0:R], lhsT2B, rhsB, start=True, stop=True)
        B2 = pool.tile([128, R], f32)
        nc.scalar.activation(B2[:], psB[:, 0:R], Act.Exp,
                             scale=-inv2sig2, bias=biasB[:, 0:1])
        G = pool.tile([128, HALF], bf16)
        nc.vector.tensor_tensor(
            G[:].rearrange("p (a b) -> p a b", a=25),
            A2[:].unsqueeze(2).broadcast_to([128, 25, R]),
            B2[:].unsqueeze(1).broadcast_to([128, 25, R]),
            Op.mult)

        # ---- per-event exponent via 2 accumulated K=2 matmuls (fp32)
        sq1 = pool.tile([1, L], f32)
        nc.vector.tensor_tensor(sq1[:], s1_row, s1_row, Op.mult)
        sq2 = pool.tile([1, L], f32)
        nc.vector.tensor_tensor(sq2[:], s2_row, s2_row, Op.mult)
        ssum = pool.tile([1, L], f32)
        nc.vector.tensor_tensor(ssum[:], sq1[:], sq2[:], Op.add)
        w2 = pool.tile([2, L], f32)
        nc.vector.memset(w2[:], 0.0)
        nc.vector.tensor_scalar(w2[0:1, :], ssum[:], -inv2sig2, None, Op.mult)

        pairA = pool.tile([2, L], f32)
        nc.vector.scalar_tensor_tensor(pairA[:], pA_in, scoefA, w2[:],
                                       Op.mult, Op.add)
        pairB = pool.tile([2, L], f32)
        nc.vector.scalar_tensor_tensor(pairB[:], pB_in, scoefB, w2[:],
                                       Op.mult, Op.add)
        ha = pool.tile([1, L], f32)
        nc.vector.tensor_scalar(ha[:], t_row, 0.0, 1.0, Op.is_gt, Op.subtract)
        hm = pool.tile([1, L], f32)
        nc.vector.tensor_scalar(hm[:], ha[:], BIG_NEG, None, Op.mult)
        nc.vector.tensor_tensor(pairB[0:1, :], pairB[0:1, :], hm[:], Op.add)

        zev_ps = psmall.tile([L, L], f32, tag="small")
        nc.tensor.matmul(zev_ps[:], pairA[:], rhs_mm1, start=True, stop=False)
        nc.tensor.matmul(zev_ps[:], lhsT_mm2, pairB[:], start=False, stop=True)

        Ke = pool.tile([L, L], f32)
        nc.scalar.activation(Ke[:], zev_ps[:], Act.Exp)
        Km = pool.tile([L, L], f32)
        lam_col = pool.tile([L, 1], f32)
        nc.vector.scalar_tensor_tensor(Km[:], Ke[:], 0.0, ctril[:],
                                       Op.add, Op.mult, accum_out=lam_col[:])

        # lams = softplus(lam_raw + mu) + 1e-5 ; log pieces (ACT side early)
        rr = pool.tile([L, 1], f32)
        nc.vector.tensor_scalar(rr[:], lam_col[:], mu, 0.0, Op.add, Op.max)
        tt_ = pool.tile([L, 1], f32)
        nc.vector.scalar_tensor_tensor(tt_[:], rr[:], 2.0, lam_col[:],
                                       Op.mult, Op.subtract)
        ee = pool.tile([L, 1], f32)
        nc.scalar.activation(ee[:], tt_[:], Act.Exp, scale=-1.0,
                             bias=mu_col[0:L, 0:1])
        lp = pool.tile([L, 1], f32)
        nc.scalar.activation(lp[:], ee[:], Act.Ln, bias=1.0)
        lams_col = pool.tile([L, 1], f32)
        nc.vector.scalar_tensor_tensor(lams_col[:], lp[:], 1e-5, rr[:],
                                       Op.add, Op.add)
        nc.sync.dma_start(lams_o.unsqueeze(1), lams_col[:])
        lnl = pool.tile([L, 1], f32)
        nc.scalar.activation(lnl[:], lams_col[:], Act.Ln)
        lnm = pool.tile([L, 1], f32)
        nc.vector.tensor_tensor(lnm[:], lnl[:], h_col[0:L, 0:1], Op.mult)

        # ---- grid: per 512-chunk pipeline of W.T@G -> softplus+accumulate
        # acc: cols 0:3 = relu sums, cols 3:6 = log1p sums
        acc = pool.tile([128, 6], f32)
        zz = pool.tile([128, 512], f32)
        nc.vector.memset(zz[:], 0.0)
        z_ps = psum.tile([128, 1536], f32, tag="z")
        for ci, (off, w) in enumerate(CHUNKS):
            for h in (0, 1):
                p0 = h * 64
                nc.tensor.matmul(z_ps[p0:p0 + 64, off:off + w],
                                 WT2[p0:p0 + 64, 0:64],
                                 G[p0:p0 + 64, off:off + w],
                                 start=True, stop=True)
            # softplus(v)+sum, v = z+mu: relu(v) and log1p(exp(-|v|)),
            # both row sums ride accum_out ports
            r_c = cpool.tile([128, 512], f32, tag="r_c")
            nc.vector.scalar_tensor_tensor(r_c[:, 0:w], z_ps[:, off:off + w],
                                           mu, zz[:, 0:w], Op.add, Op.max,
                                           accum_out=acc[:, ci:ci + 1])
            a_c = cpool.tile([128, 512], f32, tag="a_c")
            nc.scalar.activation(a_c[:, 0:w], z_ps[:, off:off + w], Act.Abs,
                                 bias=mu_col[:, 0:1])
            e_c = cpool.tile([128, 512], f32, tag="e_c")
            nc.scalar.activation(e_c[:, 0:w], a_c[:, 0:w], Act.Exp,
                                 scale=-1.0)
            l_c = cpool.tile([128, 512], f32, tag="l_c")
            nc.scalar.activation(l_c[:, 0:w], e_c[:, 0:w], Act.Ln, bias=1.0,
                                 accum_out=acc[:, 3 + ci:4 + ci])

        # ---- reductions and final combine
        int_col = pool.tile([128, 1], f32)
        nc.vector.tensor_reduce(int_col[:], acc[:, 0:6],
                                mybir.AxisListType.X, Op.add)
        red_ps = psmall.tile([1, 2], f32, tag="small")
        nc.tensor.matmul(red_ps[0:1, 0:1], lnm[:, 0:1], ones_col[0:L, 0:1],
                         start=True, stop=True)
        nc.tensor.matmul(red_ps[0:1, 1:2], int_col[:, 0:1], sel_col,
                         start=True, stop=True)

        sl_sb = pool.tile([1, 1], f32)
        nc.scalar.copy(sl_sb[:], red_ps[0:1, 0:1])
        ll_sb = pool.tile([1, 1], f32)
        nc.vector.scalar_tensor_tensor(ll_sb[:], red_ps[0:1, 1:2], -UNIT_VOL,
                                       sl_sb[:], Op.mult, Op.add)
        nc.sync.dma_start(ll_o.unsqueeze(1), ll_sb[:])

    import concourse.hw_specs as hw_specs
    orig = bacc.get_activation_tables
    bacc.get_activation_tables = _patched_act_tables(hw_specs.get_activation_tables)
    try:
        nc.compile()
    finally:
        bacc.get_activation_tables = orig
    return nc


def _get_program(mu, alpha, beta, sigma, L):
    key = (float(mu), float(alpha), float(beta), float(sigma), L)
    if key not in _prog_cache:
        sig2 = float(sigma) * float(sigma)
        inv2sig2 = 1.0 / (2.0 * sig2)
        norm = float(alpha) * float(beta) / (2.0 * math.pi * sig2)
        nc = _build_program(float(mu), float(beta), inv2sig2, norm, L)
        consts = _const_arrays(L, norm, float(beta), inv2sig2)
        _prog_cache[key] = (nc, consts)
    return _prog_cache[key]


def kernel(x, mu, alpha, beta, sigma):
    from concourse.bass_utils import run_bass_kernel_spmd

    x = np.asarray(x, dtype=np.float32)
    B, L, _ = x.shape
    assert B == NCORES, f"expected batch {NCORES}, got {B}"

    nc, consts = _get_program(mu, alpha, beta, sigma, L)

    in_maps = []
    for b in range(B):
        m = dict(consts)
        m.update(_marshal_core_inputs(x[b, :, 0], x[b, :, 1], x[b, :, 2]))
        in_maps.append(m)

    res = run_bass_kernel_spmd(nc, in_maps, list(range(NCORES)))
    lams = np.stack([res.results[b]["lams_o"] for b in range(B)]).astype(np.float32)
    loglik = np.stack([res.results[b]["ll_o"][0] for b in range(B)]).astype(np.float32)
    return lams, loglik


# revision 16
# speedup vs baseline: 1.2721x; 1.0222x over previous
"""Spatio-temporal Hawkes process log-likelihood on Trainium2 (Bass/Tile).

Computes, for x[B, L, 3] = (t, s1, s2) and scalars mu/alpha/beta/sigma:
  lams[b, i]  = softplus(sum_{j<i} K(x_i, x_j) * 1[t_j>0] + mu) + 1e-5
  loglik[b]   = sum_i log(lams[b,i]) * 1[t_i>0]
              - UNIT_VOL * sum_{r, g} softplus(sum_j K((tt_r, ss_g), x_j) * m + mu)
with K(x, y) = norm * exp(-beta*(t_x - t_y) - |s_x - s_y|^2 / (2 sigma^2)),
norm = alpha*beta/(2 pi sigma^2), over a 50 x 50 x 50 (t, s1, s2) grid.

Strategy (one batch element per NeuronCore, 8 cores, data-parallel):
  The grid kernel factorizes in time AND is separable in space:
    K((tt_r, ss_g), x_j) = [norm * 1[0<t_j<=tt_r] * e^{beta(t_j-tt_r)}]
                         * e^{-inv2sig2 (g1_i-s1_j)^2} * e^{-inv2sig2 (g2_k-s2_j)^2}
  so per core we build two tiny per-axis tables A[j,i] (25 cols/half)
  and B[j,k] (50 cols) via K<=4 quadratic-expansion matmuls + ACT exp,
  expand G[j,g] = A*B with one broadcast-AP multiply, build the
  temporal weight matrix W[j,r], and get the softplus argument as
  W.T @ G on the PE (bf16 operands, fp32 PSUM accumulation).
  softplus+row-sum is chunk-pipelined against the matmuls, with the
  row sums riding the accum_out ports of the DVE relu and ACT log1p.
  The per-event [L, L] exponent is built by 2 accumulated K=2 fp32
  matmuls over pair-packed rank-1 factors (fp32 because the quadratic
  expansion cancels catastrophically in low precision), one ACT exp,
  and a masked row-reduce fused into a scalar_tensor_tensor.

  Partition packing: the 2500 spatial grid points are split in two
  halves of 1250; partitions 0:64 hold events-vs-half0, 64:128 hold
  events-vs-half1, so elementwise engines run at full 128-lane width.

softplus is decomposed as relu(v) + log1p(exp(-|v|)) and the
activation-table map is patched during compile so every ACT func
(Exp/Ln/Abs/Copy) resolves to the single `natural_log_exp_and_others`
set -> one table load, no set thrashing.

All tiny per-core staging (dup columns, concatenated rows, packed
rank-1 factors) is marshalled host-side as pure copies - engines can
only address SBUF partition starts of 0/32/64/96, so single-row writes
at other partitions are not expressible on-device.  Emission order is
tuned so no in-order engine stream blocks on a late dependency.
"""

import math
import numpy as np
from contextlib import ExitStack

R = 50                      # INT_RES (time and each spatial axis)
RG = R * R                  # 2500 spatial grid points
HALF = RG // 2              # 1250
NCORES = 8
UNIT_VOL = 1.0 / float(R ** 3)
BIG_NEG = 1.0e30
CHUNKS = ((0, 512), (512, 512), (1024, HALF - 1024))

_prog_cache: dict = {}


def _const_arrays(L: int, norm: float, beta: float, inv2sig2: float):
    f32 = np.float32
    g1 = np.linspace(0.0, 1.0, R).astype(f32)
    g2 = np.linspace(0.0, 1.0, R).astype(f32)

    ccols = np.zeros((128, 2), f32)
    ccols[:, 0] = 1.0                                   # ones column
    ccols[:, 1] = (np.arange(128) % 64 < R)             # sel (valid r rows)

    ctril = (norm * np.tril(np.ones((L, L), np.float64), -1)).astype(f32)

    # crows [4, 256]:
    #   [0:1, 0:128]   ones row
    #   [0:1, 128:178] -linspace(0,1,R)
    #   [0:2, 178:180] per-event STT coefficient columns
    #   [0:4, 180:205] rhs of A-table matmul [g1lo^2; g1hi^2; -2g1lo; -2g1hi]
    #   [0:2, 205:255] rhs of B-table matmul [g2^2; -2g2]
    crows = np.zeros((4, 256), f32)
    crows[0, 0:128] = 1.0
    crows[0, 128:178] = -np.linspace(0.0, 1.0, R)
    crows[0, 178] = -beta; crows[1, 178] = 2.0 * inv2sig2    # colA -> [u; a1]
    crows[0, 179] = beta;  crows[1, 179] = 2.0 * inv2sig2    # colB -> [v; a2]
    g1lo, g1hi = g1[0:25], g1[25:50]
    crows[0, 180:205] = g1lo ** 2
    crows[1, 180:205] = g1hi ** 2
    crows[2, 180:205] = -2.0 * g1lo
    crows[3, 180:205] = -2.0 * g1hi
    crows[0, 205:255] = g2 ** 2
    crows[1, 205:255] = -2.0 * g2
    return dict(ccols=ccols, ctril=ctril, crows=crows)


def _marshal_core_inputs(t, s1, s2):
    """Pure-layout staging of one sequence's inputs (no arithmetic).

    icols [128, 3]: t/s1/s2 duplicated into both partition halves.
    irows [8, 704]:
      [0:1, 0:192]   t | s1 | s2 concatenated rows
      [0:4, 192:320] lhsT of the A-table matmul
                     [ind_lo; ind_hi; s1*ind_lo; s1*ind_hi]
      [0:2, 320:384] [t; s1]   (pair-packed per-event row inputs)
      [0:2, 384:448] [t; s2]
      [0:2, 448:512] [ones; s1] (rhs of per-event matmul 1)
      [0:2, 512:576] [ones; s2] (lhsT of per-event matmul 2)
      [0:2, 576:704] [ones128; s2_dup] (lhsT of the B-table matmul)
    """
    f32 = np.float32
    L = t.shape[0]
    icols = np.zeros((128, 3), f32)
    icols[0:L, 0] = t; icols[64:64 + L, 0] = t
    icols[0:L, 1] = s1; icols[64:64 + L, 1] = s1
    icols[0:L, 2] = s2; icols[64:64 + L, 2] = s2
    irows = np.zeros((8, 704), f32)
    irows[0, 0:L] = t
    irows[0, L:2 * L] = s1
    irows[0, 2 * L:3 * L] = s2
    irows[0, 192:256] = 1.0                       # ind_lo
    irows[1, 256:320] = 1.0                       # ind_hi
    irows[2, 192:192 + L] = s1
    irows[3, 256:256 + L] = s1
    irows[0, 320:320 + L] = t;   irows[1, 320:320 + L] = s1
    irows[0, 384:384 + L] = t;   irows[1, 384:384 + L] = s2
    irows[0, 448:448 + L] = 1.0; irows[1, 448:448 + L] = s1
    irows[0, 512:512 + L] = 1.0; irows[1, 512:512 + L] = s2
    irows[0, 576:704] = 1.0
    irows[1, 576:576 + L] = s2;  irows[1, 640:640 + L] = s2
    return {"icols": icols, "irows": irows}


def _patched_act_tables(orig_fn, preferred="natural_log_exp_and_others"):
    """Wrap get_activation_tables so every function present in the
    preferred set resolves only to it (same names/order, so the emitted
    act_func_set_id still indexes the real act_info.json)."""
    import functools

    @functools.cache
    def wrapper(arch):
        tables = dict(orig_fn(arch))
        pref = tables.get(preferred)
        if not pref:
            return tables
        return {
            name: (funcs if name == preferred else funcs - pref)
            for name, funcs in tables.items()
        }
    return wrapper


def _build_program(mu: float, beta: float, inv2sig2: float, norm: float, L: int):
    import concourse.bass as bass
    import concourse.bacc as bacc
    import concourse.tile as tile
    import concourse.mybir as mybir

    f32 = mybir.dt.float32
    bf16 = mybir.dt.bfloat16
    Act = mybir.ActivationFunctionType
    Op = mybir.AluOpType

    nc = bacc.Bacc("TRN2", target_bir_lowering=False, debug=False,
                   enable_asserts=True, num_devices=NCORES)

    # ---- DRAM I/O
    icols_d = nc.dram_tensor("icols", [128, 3], f32, kind="ExternalInput").ap()
    irows_d = nc.dram_tensor("irows", [8, 704], f32, kind="ExternalInput").ap()
    ccols_d = nc.dram_tensor("ccols", [128, 2], f32, kind="ExternalInput").ap()
    ctril_d = nc.dram_tensor("ctril", [L, L], f32, kind="ExternalInput").ap()
    crows_d = nc.dram_tensor("crows", [4, 256], f32, kind="ExternalInput").ap()
    lams_o = nc.dram_tensor("lams_o", [L], f32, kind="ExternalOutput").ap()
    ll_o = nc.dram_tensor("ll_o", [1], f32, kind="ExternalOutput").ap()

    with tile.TileContext(nc) as tc, ExitStack() as ctx:
        pool = ctx.enter_context(tc.tile_pool(name="sbuf", bufs=1))
        cpool = ctx.enter_context(tc.tile_pool(name="chunk", bufs=2))
        psum = ctx.enter_context(tc.tile_pool(name="psum", bufs=1,
                                              space=bass.MemorySpace.PSUM))
        psmall = ctx.enter_context(tc.tile_pool(name="psmall", bufs=3,
                                                space=bass.MemorySpace.PSUM))

        # ---- loads (5 small DMAs on 3 queues)
        irows = pool.tile([8, 704], f32)
        nc.sync.dma_start(irows[:], irows_d[:])
        icols = pool.tile([128, 3], f32)
        nc.sync.dma_start(icols[:], icols_d[:])
        crows = pool.tile([4, 256], f32)
        nc.gpsimd.dma_start(crows[:], crows_d[:])
        ctril = pool.tile([L, L], f32)
        nc.gpsimd.dma_start(ctril[:], ctril_d[:])
        ccols = pool.tile([128, 2], f32)
        nc.scalar.dma_start(ccols[:], ccols_d[:])

        t_col = icols[:, 0:1]
        s1_col = icols[:, 1:2]
        s2_col = icols[:, 2:3]
        t_row = irows[0:1, 0:L]
        s1_row = irows[0:1, L:2 * L]
        s2_row = irows[0:1, 2 * L:3 * L]
        lhsT4A = irows[0:4, 192:320]
        pA_in = irows[0:2, 320:320 + L]
        pB_in = irows[0:2, 384:384 + L]
        rhs_mm1 = irows[0:2, 448:448 + L]
        lhsT_mm2 = irows[0:2, 512:512 + L]
        lhsT2B = irows[0:2, 576:704]

        ones_col = ccols[:, 0:1]
        sel_col = ccols[:, 1:2]
        ones_r = crows[0:1, 0:128]
        ones_r64 = crows[0:1, 0:L]
        negttg = crows[0:1, 128:178]
        scoefA = crows[0:2, 178:179]
        scoefB = crows[0:2, 179:180]
        rhsA = crows[0:4, 180:205]
        rhsB = crows[0:2, 205:255]

        mu_col = pool.tile([128, 1], f32)
        nc.vector.memset(mu_col[:], mu)

        # ---- per-partition spatial biases: -inv2sig2*s1^2, -inv2sig2*s2^2
        biasA = pool.tile([128, 1], f32)
        nc.vector.tensor_scalar(biasA[:], s1_col, s1_col, -inv2sig2,
                                Op.mult, Op.mult)
        biasB = pool.tile([128, 1], f32)
        nc.vector.tensor_scalar(biasB[:], s2_col, s2_col, -inv2sig2,
                                Op.mult, Op.mult)

        # ---- temporal weights W_T[j(packed), r] (norm folded into mask)
        bc_ps = psmall.tile([128, R], f32, tag="small")
        nc.tensor.matmul(bc_ps[:], ones_r, negttg, start=True, stop=True)
        dtW = pool.tile([128, R], f32)
        nc.vector.tensor_scalar(dtW[:], bc_ps[:], t_col, None, Op.add)
        Ew = pool.tile([128, R], f32)
        nc.scalar.activation(Ew[:], dtW[:], Act.Exp, scale=beta)
        hn_col = pool.tile([128, 1], f32)
        nc.vector.tensor_scalar(hn_col[:], t_col, 0.0, norm, Op.is_gt, Op.mult)
        h_col = pool.tile([128, 1], f32)
        nc.vector.tensor_scalar(h_col[:], t_col, 0.0, None, Op.is_gt)
        Mw = pool.tile([128, R], f32)
        nc.vector.tensor_scalar(Mw[:], dtW[:], 0.0, hn_col[:, 0:1], Op.is_le, Op.mult)
        WT2 = pool.tile([128, 64], bf16)
        nc.vector.memset(WT2[:], 0.0)
        nc.vector.tensor_tensor(WT2[:, 0:R], Ew[:], Mw[:], Op.mult)

        # ---- separable spatial kernel: G[p, i*50+k] = A[p, i] * B[p, k]
        psA = psmall.tile([128, 32], f32, tag="small")
        nc.tensor.matmul(psA[:, 0:25], lhsT4A, rhsA, start=True, stop=True)
        A2 = pool.tile([128, 25], f32)
        nc.scalar.activation(A2[:], psA[:, 0:25], Act.Exp,
                             scale=-inv2sig2, bias=biasA[:, 0:1])
        psB = psmall.tile([128, 64], f32, tag="small")
        nc.tensor.matmul(psB[:, 0:R], lhsT2B, rhsB, start=True, stop=True)
        B2 = pool.tile([128, R], f32)
        nc.scalar.activation(B2[:], psB[:, 0:R], Act.Exp,
                             scale=-inv2sig2, bias=biasB[:, 0:1])
        G = pool.tile([128, HALF], bf16)
        nc.vector.tensor_tensor(
            G[:].rearrange("p (a b) -> p a b", a=25),
            A2[:].unsqueeze(2).broadcast_to([128, 25, R]),
            B2[:].unsqueeze(1).broadcast_to([128, 25, R]),
            Op.mult)

        # ---- per-event exponent via 2 accumulated K=2 matmuls (fp32)
        sq1 = pool.tile([1, L], f32)
        nc.vector.tensor_tensor(sq1[:], s1_row, s1_row, Op.mult)
        sq2 = pool.tile([1, L], f32)
        nc.vector.tensor_tensor(sq2[:], s2_row, s2_row, Op.mult)
        ssum = pool.tile([1, L], f32)
        nc.vector.tensor_tensor(ssum[:], sq1[:], sq2[:], Op.add)
        w2 = pool.tile([2, L], f32)
        nc.vector.memset(w2[:], 0.0)
        nc.vector.tensor_scalar(w2[0:1, :], ssum[:], -inv2sig2, None, Op.mult)

        pairA = pool.tile([2, L], f32)
        nc.vector.scalar_tensor_tensor(pairA[:], pA_in, scoefA, w2[:],
                                       Op.mult, Op.add)
        pairB = pool.tile([2, L], f32)
        nc.vector.scalar_tensor_tensor(pairB[:], pB_in, scoefB, w2[:],
                                       Op.mult, Op.add)
        ha = pool.tile([1, L], f32)
        nc.vector.tensor_scalar(ha[:], t_row, 0.0, 1.0, Op.is_gt, Op.subtract)
        hm = pool.tile([1, L], f32)
        nc.vector.tensor_scalar(hm[:], ha[:], BIG_NEG, None, Op.mult)
        nc.vector.tensor_tensor(pairB[0:1, :], pairB[0:1, :], hm[:], Op.add)

        zev_ps = psmall.tile([L, L], f32, tag="small")
        nc.tensor.matmul(zev_ps[:], pairA[:], rhs_mm1, start=True, stop=False)
        nc.tensor.matmul(zev_ps[:], lhsT_mm2, pairB[:], start=False, stop=True)

        Ke = pool.tile([L, L], f32)
        nc.scalar.activation(Ke[:], zev_ps[:], Act.Exp)
        Km = pool.tile([L, L], f32)
        lam_col = pool.tile([L, 1], f32)
        nc.vector.scalar_tensor_tensor(Km[:], Ke[:], 0.0, ctril[:],
                                       Op.add, Op.mult, accum_out=lam_col[:])

        # lams = softplus(lam_raw + mu) + 1e-5 ; log pieces (ACT side early)
        rr = pool.tile([L, 1], f32)
        nc.vector.tensor_scalar(rr[:], lam_col[:], mu, 0.0, Op.add, Op.max)
        tt_ = pool.tile([L, 1], f32)
        nc.vector.scalar_tensor_tensor(tt_[:], rr[:], 2.0, lam_col[:],
                                       Op.mult, Op.subtract)
        ee = pool.tile([L, 1], f32)
        nc.scalar.activation(ee[:], tt_[:], Act.Exp, scale=-1.0,
                             bias=mu_col[0:L, 0:1])
        lp = pool.tile([L, 1], f32)
        nc.scalar.activation(lp[:], ee[:], Act.Ln, bias=1.0)
        lams_col = pool.tile([L, 1], f32)
        nc.vector.scalar_tensor_tensor(lams_col[:], lp[:], 1e-5, rr[:],
                                       Op.add, Op.add)
        nc.sync.dma_start(lams_o.unsqueeze(1), lams_col[:])
        lnl = pool.tile([L, 1], f32)
        nc.scalar.activation(lnl[:], lams_col[:], Act.Ln)
        lnm = pool.tile([L, 1], f32)
        nc.vector.tensor_tensor(lnm[:], lnl[:], h_col[0:L, 0:1], Op.mult)

        # ---- grid: matmuls per 512 PSUM bank; softplus pipelined in two
        #      625-wide chunks (bank-level deps let them overlap the mms).
        # acc: cols 0:2 = relu sums, cols 2:4 = log1p sums
        acc = pool.tile([128, 4], f32)
        zz = pool.tile([128, 625], f32)
        nc.vector.memset(zz[:], 0.0)
        z_ps = psum.tile([128, 1536], f32, tag="z")
        SP_CHUNKS = ((0, 625), (625, 625))
        emitted = set()

        def _mms_for(lo, hi):
            for moff, mw in CHUNKS:
                if moff in emitted or moff >= hi or moff + mw <= lo:
                    continue
                emitted.add(moff)
                for h in (0, 1):
                    p0 = h * 64
                    nc.tensor.matmul(z_ps[p0:p0 + 64, moff:moff + mw],
                                     WT2[p0:p0 + 64, 0:64],
                                     G[p0:p0 + 64, moff:moff + mw],
                                     start=True, stop=True)

        for ci, (off, w) in enumerate(SP_CHUNKS):
            _mms_for(off, off + w)
            r_c = cpool.tile([128, 640], f32, tag="r_c")
            nc.vector.scalar_tensor_tensor(r_c[:, 0:w], z_ps[:, off:off + w],
                                           mu, zz[:, 0:w], Op.add, Op.max,
                                           accum_out=acc[:, ci:ci + 1])
            t_c = cpool.tile([128, 640], f32, tag="t_c")
            nc.vector.scalar_tensor_tensor(t_c[:, 0:w], r_c[:, 0:w], 2.0,
                                           z_ps[:, off:off + w],
                                           Op.mult, Op.subtract)
            e_c = cpool.tile([128, 640], f32, tag="e_c")
            nc.scalar.activation(e_c[:, 0:w], t_c[:, 0:w], Act.Exp,
                                 scale=-1.0, bias=mu_col[:, 0:1])
            l_c = cpool.tile([128, 640], f32, tag="l_c")
            nc.scalar.activation(l_c[:, 0:w], e_c[:, 0:w], Act.Ln, bias=1.0,
                                 accum_out=acc[:, 2 + ci:3 + ci])

        # ---- reductions and final combine
        int_col = pool.tile([128, 1], f32)
        nc.vector.tensor_reduce(int_col[:], acc[:, 0:4],
                                mybir.AxisListType.X, Op.add)
        red_ps = psmall.tile([1, 2], f32, tag="small")
        nc.tensor.matmul(red_ps[0:1, 0:1], lnm[:, 0:1], ones_col[0:L, 0:1],
                         start=True, stop=True)
        nc.tensor.matmul(red_ps[0:1, 1:2], int_col[:, 0:1], sel_col,
                         start=True, stop=True)

        sl_sb = pool.tile([1, 1], f32)
        nc.vector.tensor_copy(sl_sb[:], red_ps[0:1, 0:1])
        ll_sb = pool.tile([1, 1], f32)
        nc.vector.scalar_tensor_tensor(ll_sb[:], red_ps[0:1, 1:2], -UNIT_VOL,
                                       sl_sb[:], Op.mult, Op.add)
        nc.sync.dma_start(ll_o.unsqueeze(1), ll_sb[:])

    import concourse.hw_specs as hw_specs
    orig = bacc.get_activation_tables
    bacc.get_activation_tables = _patched_act_tables(hw_specs.get_activation_tables)
    try:
        nc.compile()
    finally:
        bacc.get_activation_tables = orig
    return nc


def _get_program(mu, alpha, beta, sigma, L):
    key = (float(mu), float(alpha), float(beta), float(sigma), L)
    if key not in _prog_cache:
        sig2 = float(sigma) * float(sigma)
        inv2sig2 = 1.0 / (2.0 * sig2)
        norm = float(alpha) * float(beta) / (2.0 * math.pi * sig2)
        nc = _build_program(float(mu), float(beta), inv2sig2, norm, L)
        consts = _const_arrays(L, norm, float(beta), inv2sig2)
        _prog_cache[key] = (nc, consts)
    return _prog_cache[key]


def kernel(x, mu, alpha, beta, sigma):
    from concourse.bass_utils import run_bass_kernel_spmd

    x = np.asarray(x, dtype=np.float32)
    B, L, _ = x.shape
    assert B == NCORES, f"expected batch {NCORES}, got {B}"

    nc, consts = _get_program(mu, alpha, beta, sigma, L)

    in_maps = []
    for b in range(B):
        m = dict(consts)
        m.update(_marshal_core_inputs(x[b, :, 0], x[b, :, 1], x[b, :, 2]))
        in_maps.append(m)

    res = run_bass_kernel_spmd(nc, in_maps, list(range(NCORES)))
    lams = np.stack([res.results[b]["lams_o"] for b in range(B)]).astype(np.float32)
    loglik = np.stack([res.results[b]["ll_o"][0] for b in range(B)]).astype(np.float32)
    return lams, loglik
